# revision 16
# baseline (speedup 1.0000x reference)
"""CapsuleLayer kernel for 8 Trainium2 NeuronCores (self-contained).

Strategy (v2):
  Phase 1 (data-parallel over batch, 16 examples/core):
    - primary-capsule conv as 648 f32r matmuls (81 kernel shifts x 2 cin tiles
      x 2 outch tiles x 2 batch-halves), PSUM-accumulated. PSUM free layout is
      (batch, spatial) so the resharded x is directly DMA-able.
    - bias add + squash over the 8 capsules (capsule index lives on the
      partition axis, summed via a constant selection matmul); output fp16.
  Reshard (AllToAll, fp16): x moves from batch-sharded to route-sharded with
      zero DRAM bounce copies: 2 SBUF->DRAM staging DMAs, A2A, 1 DRAM->SBUF
      load into xT [strips=(ocl,i), b, q].
  Phase 2 (route-parallel routing, fp16 elementwise):
    - pred[b, o, c, r] via 144 K=8 matmuls (4-way row-packed); copied
      PSUM->SBUF into an o-sliceable layout so routing needs no broadcasts.
    - s_hat = sum_r e*pred: 16 full-width contiguous fp16 muls (DVE 2x mode)
      + in-place binary-tree adds over r + one small fp32 tensor_reduce.
    - logits update: per-(c,o) fused scalar_tensor_tensor chains
      (out = pred*v_col + running) -- no broadcasts, no big reduces.
    - one fused AllReduce of [s_hat | Z] (128x170 fp32) per iteration.
"""

import numpy as np

B = 128
NCORES = 8
BLOC = B // NCORES          # 16
CIN = 256
KCAP = 8                    # capsules (i)
OCAP = 32                   # out channels per capsule
OUTCH = KCAP * OCAP         # 256
HW_IN = 20
KH = 9
OH = 6
S = OH * OH                 # 36 spatial positions
R = OCAP * S                # 1152 routes
RLOC = R // NCORES          # 144 routes per core
OCL = OCAP // NCORES        # 4 "oc" channels per core
C = 10                      # classes
O16 = 16                    # routing output dim
CO = C * O16                # 160
NSHIFT = KH * KH            # 81
NITER = 3
NHALF = BLOC * S // 2       # 288 (batch-half x spatial)

GP_MUL_SPLIT = 13           # o-slices [0,13) on vector, [13,16) on gpsimd
GP_DOT_SPLIT = 10           # classes [0,GP_DOT_SPLIT) on vector, rest on gpsimd
                            # (scalar_tensor_tensor only lowers on DVE)

_CACHE = {}


def _to_f32r(x):
    u = np.ascontiguousarray(x, dtype=np.float32).view(np.uint32)
    u = ((u.astype(np.uint64) + (1 << 11)) & 0xFFFFF000).astype(np.uint32)
    return u.view(np.float32)


def _build_program(profile=False, stop_stage=99):
    import concourse.bass as bass
    import concourse.tile as tile
    import concourse.mybir as mybir
    from concourse import bacc
    from contextlib import ExitStack

    dt = mybir.dt
    F16 = dt.bfloat16
    alu = mybir.AluOpType

    nc = bacc.Bacc("TRN2", target_bir_lowering=False, debug=False,
                   num_devices=1 if profile else NCORES)

    inp_d = nc.dram_tensor("inp", [BLOC, CIN, HW_IN, HW_IN], dt.float32r,
                           kind="ExternalInput").ap()
    wconv_d = nc.dram_tensor("wconv", [NSHIFT, 2, 128, OUTCH], dt.float32r,
                             kind="ExternalInput").ap()
    bias_d = nc.dram_tensor("bias", [2, 128], dt.float32,
                            kind="ExternalInput").ap()
    wroute_d = nc.dram_tensor("wroute", [OCL, KCAP, S, CO], F16,
                              kind="ExternalInput").ap()
    sel8_d = nc.dram_tensor("sel8", [128, 32], dt.float32,
                            kind="ExternalInput").ap()
    sel32_d = nc.dram_tensor("sel32", [32, 128], dt.float32,
                             kind="ExternalInput").ap()
    ones_d = nc.dram_tensor("ones", [128, 1], dt.float32,
                            kind="ExternalInput").ap()
    onesr_d = nc.dram_tensor("onesr", [1, 128], dt.float32,
                             kind="ExternalInput").ap()
    vout_d = nc.dram_tensor("vout", [B, CO], dt.float32,
                            kind="ExternalOutput").ap()

    RGRP = [list(range(NCORES))]

    def _emit():
      with tile.TileContext(nc) as tc, ExitStack() as top:
          dram = top.enter_context(tc.tile_pool(name="dram", bufs=1, space="DRAM"))
          routing = top.enter_context(tc.tile_pool(name="routing", bufs=1))

          # A2A chunk j -> dest core j; content [ocl, i, b, s] fp16
          a2a_in = dram.tile([NCORES, OCL, KCAP, BLOC, S], F16, tag="a2ai")
          a2a_out = dram.tile([NCORES, OCL, KCAP, BLOC, S], F16, tag="a2ao")

          # phase-2 statics, loaded early (hidden under conv)
          # xT: [32*ocl + i | b, q], rows 8..31 of each strip zero
          xT = routing.tile([128, B, S], F16, tag="xT")
          wr = routing.tile([128, S, CO], F16, tag="wr")
          onesb = routing.tile([128, 1], dt.float32, tag="onesb")
          onesr = routing.tile([1, 128], dt.float32, tag="onesr")
          nc.vector.memset(xT, 0.0)
          nc.vector.memset(wr, 0.0)
          for rg in range(OCL):
              nc.sync.dma_start(out=wr[32 * rg:32 * rg + KCAP, :, :],
                                in_=wroute_d[rg])
          nc.sync.dma_start(out=onesb, in_=ones_d)
          nc.sync.dma_start(out=onesr, in_=onesr_d)

          # ---------------- Phase 1: conv + squash ----------------
          with ExitStack() as conv_scope:
              cpool = conv_scope.enter_context(tc.tile_pool(name="conv", bufs=1))
              wpool = conv_scope.enter_context(tc.tile_pool(name="wstream", bufs=4))
              cps = conv_scope.enter_context(
                  tc.tile_pool(name="cpsum", bufs=1, space="PSUM"))

              it = []
              for ct in range(2):
                  t = cpool.tile([128, BLOC * 400], dt.float32r, tag=f"in{ct}",
                                 name=f"in{ct}")
                  nc.sync.dma_start(
                      out=t[:].rearrange("ci (b f) -> ci b f", b=BLOC),
                      in_=inp_d.rearrange("b (ct ci) h w -> ct ci b (h w)", ct=2)[ct])
                  it.append(t)

              cp = [cps.tile([128, 2, 512], dt.float32, tag=f"cp{ot}", name=f"cp{ot}")
                    for ot in range(2)]

              for si in range(NSHIFT):
                  dh, dw = divmod(si, KH)
                  for ct in range(2):
                      wt = wpool.tile([128, OUTCH], dt.float32r, tag="w")
                      nc.sync.dma_start(out=wt, in_=wconv_d[si, ct])
                      for ot in range(2):
                          lhsT = wt[:, 128 * ot:128 * (ot + 1)]
                          for bh in range(2):
                              rhs = it[ct][:].rearrange(
                                  "ci (b h w) -> ci b h w", b=BLOC, h=HW_IN)[
                                  :, 8 * bh:8 * bh + 8,
                                  dh:dh + 12:2, dw:dw + 12:2]
                              nc.tensor.matmul(
                                  cp[ot][0:128, bh, 0:NHALF], lhsT, rhs,
                                  start=(si == 0 and ct == 0),
                                  stop=(si == NSHIFT - 1 and ct == 1))

              # bias add (PSUM -> SBUF); free layout (b16, s36)
              bt = cpool.tile([128, 2], dt.float32, tag="bias")
              nc.sync.dma_start(out=bt[:], in_=bias_d.rearrange("t p -> p t"))
              xsb = []
              for ot in range(2):
                  x = cpool.tile([128, 2 * NHALF], dt.float32, tag=f"x{ot}",
                                 name=f"x{ot}")
                  for bh in range(2):
                      nc.vector.tensor_scalar_add(
                          out=x[:, NHALF * bh:NHALF * (bh + 1)],
                          in0=cp[ot][0:128, bh, 0:NHALF],
                          scalar1=bt[:, ot:ot + 1])
                  xsb.append(x)

              # squash over capsule axis (partition groups of 32)
              sel8 = cpool.tile([128, 32], dt.float32, tag="sel8")
              sel32 = cpool.tile([32, 128], dt.float32, tag="sel32")
              nc.sync.dma_start(out=sel8, in_=sel8_d)
              nc.sync.dma_start(out=sel32, in_=sel32_d)

              sq = cpool.tile([128, 2 * NHALF], dt.float32, tag="sq")
              snp = [cps.tile([32, NHALF], dt.float32, tag=f"snp{h}", name=f"snp{h}")
                     for h in range(2)]
              for ot in range(2):
                  nc.scalar.square(out=sq, in_=xsb[ot][:])
                  for h in range(2):
                      nc.tensor.matmul(
                          snp[h][0:32, 0:NHALF], sel8[:],
                          sq[:, NHALF * h:NHALF * (h + 1)],
                          start=(ot == 0), stop=(ot == 1))
              # g = sqrt(sn) / (1 + sn)
              g = cpool.tile([32, 2 * NHALF], dt.float32, tag="g")
              gtmp = cpool.tile([32, 2 * NHALF], dt.float32, tag="gtmp")
              for h in range(2):
                  hs = slice(NHALF * h, NHALF * (h + 1))
                  nc.scalar.sqrt(out=g[:, hs], in_=snp[h][0:32, 0:NHALF])
                  nc.vector.tensor_scalar_add(out=gtmp[:, hs],
                                              in0=snp[h][0:32, 0:NHALF],
                                              scalar1=1.0)
              nc.vector.reciprocal(out=gtmp, in_=gtmp)
              nc.vector.tensor_mul(out=g, in0=g, in1=gtmp)
              # replicate g across the 8 capsule partition groups
              grp = [cps.tile([128, NHALF], dt.float32, tag=f"grp{h}", name=f"grp{h}")
                     for h in range(2)]
              for h in range(2):
                  nc.tensor.matmul(grp[h][0:128, 0:NHALF], sel32[:],
                                   g[:, NHALF * h:NHALF * (h + 1)],
                                   start=True, stop=True)
              # x * g, cast fp16
              xsb2 = []
              for ot in range(2):
                  x2 = cpool.tile([128, 2 * NHALF], F16, tag=f"x2{ot}",
                                  name=f"x2{ot}")
                  for h in range(2):
                      hs = slice(NHALF * h, NHALF * (h + 1))
                      nc.vector.tensor_mul(out=x2[:, hs], in0=xsb[ot][:, hs],
                                           in1=grp[h][0:128, 0:NHALF])
                  xsb2.append(x2)

              # stage directly into the A2A input: partition p = 32*i2 + 4*j + ocl
              # (barrier: Tile's tracker misses deps through the
              #  partition-decomposed rearrange APs below)
              tc.strict_bb_all_engine_barrier()
              for ot in range(2):
                  for i2 in range(4):
                      nc.sync.dma_start(
                          out=a2a_in[:, :, 4 * ot + i2].rearrange(
                              "j ocl b s -> (j ocl) (b s)"),
                          in_=xsb2[ot][32 * i2:32 * i2 + 32, :])

          if stop_stage == 1:
              pr1 = routing.tile([128, 2 * NHALF], dt.float32, tag="probe1")
              nc.vector.tensor_copy(out=pr1, in_=xsb2[0][:])
              nc.sync.dma_start(out=vout_d, in_=pr1[:, 0:CO])
              return

          if stop_stage == 15:
              # read back a2a_in[0]: [ocl, i, b 0:4, s] -> [32, 144]
              prb = routing.tile([32, 144], F16, tag="probe15b")
              pr15 = routing.tile([128, CO], dt.float32, tag="probe15")
              tc.strict_bb_all_engine_barrier()
              nc.sync.dma_start(
                  out=prb[:].rearrange("(ocl i) (b s) -> ocl i b s",
                                       ocl=OCL, b=4),
                  in_=a2a_in[0, :, :, 0:4, :])
              nc.vector.memset(pr15, 0.0)
              nc.vector.tensor_copy(out=pr15[0:32, 0:144], in_=prb[:])
              nc.sync.dma_start(out=vout_d, in_=pr15[:])
              return

          # ---------------- Reshard ----------------
          tc.strict_bb_all_engine_barrier()
          if profile:
              nc.sync.dma_start(out=a2a_out[:], in_=a2a_in[:])
          else:
              nc.gpsimd.collective_compute(
                  "AllToAll", mybir.AluOpType.bypass,
                  replica_groups=RGRP, ins=[a2a_in.opt()], outs=[a2a_out.opt()])
          for rg in range(OCL):
              nc.sync.dma_start(
                  out=xT[32 * rg:32 * rg + KCAP, :, :].rearrange(
                      "i (jj b) q -> i jj b q", jj=NCORES),
                  in_=a2a_out[:, rg].rearrange("jj i b s -> i jj b s"))
          tc.strict_bb_all_engine_barrier()
          if stop_stage == 2:
              pr2 = routing.tile([128, CO], dt.float32, tag="probe2")
              nc.vector.tensor_copy(
                  out=pr2[:, 0:144].rearrange("p (b q) -> p b q", b=4),
                  in_=xT[:, 0:4, :])
              nc.vector.memset(pr2[:, 144:CO], 0.0)
              nc.sync.dma_start(out=vout_d, in_=pr2[:])
              return

          # ---------------- pred generation ----------------
          # PRED[b, o, c, r] fp16, r = rg*36 + q
          PRED = routing.tile([128, O16, C, RLOC], F16, tag="pred")
          P5 = PRED[:].rearrange("p o c (rg q) -> p o c rg q", rg=OCL)
          with ExitStack() as gen_scope:
              gps = gen_scope.enter_context(
                  tc.tile_pool(name="gpsum", bufs=2, space="PSUM"))
              for q in range(S):
                  pp = gps.tile([128, OCL, 512], dt.float32, tag="pp")
                  for rg in range(OCL):
                      nc.tensor.matmul(pp[0:128, rg, 0:CO],
                                       xT[32 * rg:32 * rg + KCAP, :, q],
                                       wr[32 * rg:32 * rg + KCAP, q, :],
                                       start=True, stop=True,
                                       tile_position=(32 * rg, 0))
                  src = pp[:, :, 0:CO].rearrange("p rg (o c) -> p o rg c", o=O16)
                  dst = P5[:, :, :, :, q].rearrange("p o c rg -> p o rg c")
                  if q % 2 == 0:
                      nc.vector.tensor_copy(out=dst, in_=src)
                  else:
                      nc.scalar.copy(out=dst, in_=src)

          if stop_stage == 3:
              pr3 = routing.tile([128, CO], dt.float32, tag="probe3")
              nc.vector.tensor_copy(
                  out=pr3[:].rearrange("p (c r) -> p c r", c=C),
                  in_=PRED[:, 0, :, 0:16])
              nc.sync.dma_start(out=vout_d, in_=pr3[:])
              return

          # ---------------- routing iterations ----------------
          tc.strict_bb_all_engine_barrier()
          logits = routing.tile([128, C, RLOC], F16, tag="logits")
          e_t = routing.tile([128, C, RLOC], F16, tag="e")
          T = routing.tile([128, O16, C, RLOC], F16, tag="T")
          dc = routing.tile([128, C, RLOC], F16, tag="dc")
          arpack = routing.tile([128, CO + C], dt.float32, tag="arpack")
          s_sb = routing.tile([128, CO + C], dt.float32, tag="s_sb")
          sqs = routing.tile([128, CO], dt.float32, tag="sqs")
          gt1 = routing.tile([1, CO], dt.float32, tag="gt1")
          gt2 = routing.tile([1, CO], dt.float32, tag="gt2")
          vsb = routing.tile([128, CO], dt.float32, tag="vsb")
          rz = routing.tile([128, C], dt.float32, tag="rz")

          with ExitStack() as it_scope:
              ips = it_scope.enter_context(
                  tc.tile_pool(name="ipsum", bufs=1, space="PSUM"))
              s0p = ips.tile([128, 512], dt.float32, tag="s0p")
              snb = ips.tile([1, CO], dt.float32, tag="snb")
              gbp = ips.tile([128, CO], dt.float32, tag="gbp")

              for t in range(NITER):
                  if t == 0:
                      # s0 = sum_r pred (uniform routing weights), exact in fp32
                      for q in range(S):
                          nc.tensor.matmul(s0p[0:128, 0:CO],
                                           xT[:, :, q], wr[:, q, :],
                                           start=(q == 0), stop=(q == S - 1))
                      nc.scalar.mul(out=arpack[:, 0:CO],
                                    in_=s0p[0:128, 0:CO], mul=1.0 / R)
                      nc.vector.memset(arpack[:, CO:CO + C], 1.0 / NCORES)
                  else:
                      # e = exp(logits), Z = sum_r e, s_hat = sum_r e * pred
                      nc.scalar.activation(out=e_t, in_=logits[:],
                                           func=mybir.ActivationFunctionType.Exp)
                      nc.vector.tensor_reduce(
                          out=arpack[:, CO:CO + C].rearrange(
                              "p (c u) -> p c u", c=C),
                          in_=e_t[:], axis=mybir.AxisListType.X,
                          op=alu.add)
                      EV = e_t[:].rearrange("p c r -> p (c r)")
                      for o in range(O16):
                          eng = nc.vector if o < GP_MUL_SPLIT else nc.gpsimd
                          eng.tensor_mul(
                              out=T[:, o].rearrange("p c r -> p (c r)"),
                              in0=PRED[:, o].rearrange("p c r -> p (c r)"),
                              in1=EV)
                      # in-place tree reduction over r: 144->72->36->18->9
                      w = RLOC
                      while w > 9:
                          hw_ = w // 2
                          nc.vector.tensor_add(
                              out=T[:, 0:GP_MUL_SPLIT, :, 0:hw_],
                              in0=T[:, 0:GP_MUL_SPLIT, :, 0:hw_],
                              in1=T[:, 0:GP_MUL_SPLIT, :, hw_:w])
                          nc.gpsimd.tensor_add(
                              out=T[:, GP_MUL_SPLIT:O16, :, 0:hw_],
                              in0=T[:, GP_MUL_SPLIT:O16, :, 0:hw_],
                              in1=T[:, GP_MUL_SPLIT:O16, :, hw_:w])
                          w = hw_
                      nc.vector.tensor_reduce(
                          out=arpack[:, 0:CO].rearrange("p (oc u) -> p oc u", u=1),
                          in_=T[:, :, :, 0:9].rearrange("p o c r -> p (o c) r"),
                          axis=mybir.AxisListType.X, op=alu.add)

                  ar_in = dram.tile([128, CO + C], dt.float32, tag="arin")
                  ar_out = dram.tile([128, CO + C], dt.float32, tag="arout")
                  nc.sync.dma_start(out=ar_in[:], in_=arpack[:])
                  if profile:
                      nc.sync.dma_start(out=ar_out[:], in_=ar_in[:])
                  else:
                      nc.gpsimd.collective_compute(
                          "AllReduce", alu.add, replica_groups=RGRP,
                          ins=[ar_in.opt()], outs=[ar_out.opt()])
                  nc.sync.dma_start(out=s_sb, in_=ar_out[:])

                  # s = s_hat / Z
                  nc.vector.reciprocal(out=rz, in_=s_sb[:, CO:CO + C])
                  sv = s_sb[:, 0:CO].rearrange("p (o c) -> p o c", o=O16)
                  nc.vector.tensor_mul(
                      out=sv, in0=sv,
                      in1=rz[:].unsqueeze(1).broadcast_to((128, O16, C)))

                  # v = squash(s) over the (full, core-local) batch axis
                  nc.scalar.square(out=sqs, in_=s_sb[:, 0:CO])
                  nc.tensor.matmul(snb[0:1, 0:CO], onesb[:], sqs[:],
                                   start=True, stop=True)
                  nc.scalar.sqrt(out=gt1, in_=snb[0:1, 0:CO])
                  nc.vector.tensor_scalar_add(out=gt2, in0=snb[0:1, 0:CO],
                                              scalar1=1.0)
                  nc.vector.reciprocal(out=gt2, in_=gt2)
                  nc.vector.tensor_mul(out=gt1, in0=gt1, in1=gt2)
                  nc.tensor.matmul(gbp[0:128, 0:CO], onesr[0:1, :], gt1[0:1, :],
                                   start=True, stop=True)
                  nc.vector.tensor_mul(out=vsb, in0=s_sb[:, 0:CO],
                                       in1=gbp[0:128, 0:CO])

                  if t < NITER - 1:
                      # logits += sum_o pred[:,o,c,:] * v[:,o*C+c]
                      for c in range(C):
                          eng = nc.vector if c < GP_DOT_SPLIT else nc.gpsimd
                          tgt = logits[:, c, :] if t == 0 else dc[:, c, :]
                          eng.tensor_scalar_mul(
                              out=tgt, in0=PRED[:, 0, c, :],
                              scalar1=vsb[:, c:c + 1])
                          for o in range(1, O16):
                              eng.scalar_tensor_tensor(
                                  out=tgt, in0=PRED[:, o, c, :],
                                  scalar=vsb[:, o * C + c:o * C + c + 1],
                                  in1=tgt, op0=alu.mult, op1=alu.add)
                          if t > 0:
                              eng.tensor_add(out=logits[:, c, :],
                                             in0=logits[:, c, :],
                                             in1=dc[:, c, :])

              nc.sync.dma_start(out=vout_d, in_=vsb[:])

    _emit()
    nc.compile()
    return nc


def _host_prep(inputs, conv_w, conv_b, route_weights):
    inputs = np.ascontiguousarray(inputs, dtype=np.float32)
    conv_w = np.ascontiguousarray(conv_w, dtype=np.float32)
    conv_b = np.ascontiguousarray(conv_b, dtype=np.float32)
    route_weights = np.ascontiguousarray(route_weights, dtype=np.float32)

    # conv weights -> [81, 2, 128, 256] (shift, cin_t, cin, outch=32k+oc)
    w = conv_w.reshape(OUTCH, CIN, KH, KH)          # [256 outch, 256 cin, 9, 9]
    w = w.transpose(2, 3, 1, 0).reshape(NSHIFT, 2, 128, OUTCH)
    wconv = _to_f32r(w)
    bias = conv_b.reshape(2, 128)

    # route weights per core: [ocl, i, s, o*C+c] fp16, oc = 4*core + ocl
    rw = route_weights.reshape(C, OCAP, S, KCAP, O16)   # [c, oc, s, i, o]
    wroute = []
    for core in range(NCORES):
        blk = rw[:, 4 * core:4 * core + OCL]            # [c, ocl, s, i, o]
        blk = blk.transpose(1, 3, 2, 4, 0).reshape(OCL, KCAP, S, CO)
        import ml_dtypes
        wroute.append(np.ascontiguousarray(blk.astype(ml_dtypes.bfloat16)))

    pidx = np.arange(128)
    sel8 = (pidx[:, None] % 32 == np.arange(32)[None, :]).astype(np.float32)
    sel32 = (np.arange(32)[:, None] == pidx[None, :] % 32).astype(np.float32)

    in_maps = []
    for core in range(NCORES):
        in_maps.append({
            "inp": _to_f32r(inputs[BLOC * core:BLOC * (core + 1)]),
            "wconv": wconv,
            "bias": bias,
            "wroute": wroute[core],
            "sel8": sel8,
            "sel32": sel32,
            "ones": np.ones((128, 1), dtype=np.float32),
            "onesr": np.ones((1, 128), dtype=np.float32),
        })
    return in_maps


def kernel(inputs, conv_w, conv_b, route_weights):
    from concourse.bass_utils import run_bass_kernel_spmd

    if "nc" not in _CACHE:
        _CACHE["nc"] = _build_program()
    nc = _CACHE["nc"]

    in_maps = _host_prep(inputs, conv_w, conv_b, route_weights)
    res = run_bass_kernel_spmd(nc, in_maps, core_ids=list(range(NCORES)))
    v = res.results[0]["vout"]                      # [128, 160] = (b, (o c))
    v = v.reshape(B, O16, C).transpose(0, 2, 1)     # -> [B, C, O16]
    return np.ascontiguousarray(v, dtype=np.float32)
